# revision 7
# baseline (speedup 1.0000x reference)
"""Trainium2 Bass kernel for nn_MinGRUStack (B=8, T=2048, Din=D=Dout=64, P=16).

Strategy (data-parallel over B, one sequence per NeuronCore, SBUF-resident):

  Each APL layer `out[n,o] = sum_i pwl_interp(t[n,i]; v[i,:,o])` is rewritten
  exactly in the relu-kink basis:
      f(t) = v0 + s0*t + sum_{k=1..14} (s_k - s_{k-1}) * relu(t - k),  s_p = v[p+1]-v[p]
  so each APL becomes a K=(1 + 14)*64 matmul on the PE:
      u^T = lin^T @ t^T + sum_c kink_c^T @ relu(stack2 - q_c)  (+ beta via eviction bias)
  where stack2 = [t^T; t^T] (128 partitions: two copies of t^T) lets one
  128-partition tensor_scalar/activation op build two relu chunks at once.
  z and h_bar share the same basis tensors: their tables are stacked into one
  M=128 matmul ([z outs | h outs]).

  The MinGRU recurrence h[t] = (1-z)h[t-1] + z*h_bar runs as a native DVE
  tensor_tensor_scan (stable direct form; reference's closed form is the same
  recurrence in exact arithmetic). max-abs normalization + the next layer's
  t = (clip(hn)+1)*7.5 fold into one per-window tensor_scalar using
  r2 = 7.5/(s+eps)  (clip is a no-op since |h/s| <= 1).

  Table preprocessing (second differences of the tiny value tables, layout
  packing) happens on the host; the device reads the same table bytes.
"""

import numpy as np

import concourse.bass as bass
import concourse.bacc as bacc
from concourse import mybir
from concourse.tile import TileContext
from concourse.bass_utils import run_bass_kernel_spmd

F32 = mybir.dt.float32
Alu = mybir.AluOpType
Act = mybir.ActivationFunctionType

T, D, NCORES = 2048, 64, 8
NW, WIN = 4, 512          # matmul windows
NT = T // 128             # 16 transpose chunks
EPS = 1e-6


# ---------------------------------------------------------------- host tables
def _parts(v):
    v = np.asarray(v, np.float32)
    beta = v[:, 0, :].sum(0)                      # (64,)
    b1 = v[:, 1, :] - v[:, 0, :]                  # (64, 64)
    s = v[:, 1:, :] - v[:, :-1, :]                # (64, 15, 64)
    c = s[:, 1:, :] - s[:, :-1, :]                # (64, 14, 64)  k=1..14
    return beta.astype(np.float32), b1.astype(np.float32), c.astype(np.float32)


def _layer_tables(vz, vh):
    bz, b1z, cz = _parts(vz)
    bh, b1h, ch = _parts(vh)
    lin = np.concatenate([b1z, b1h], axis=1)      # (64, 128)
    kinks = np.zeros((7, 128, 128), np.float32)
    for c in range(7):
        kinks[c, 0:64, 0:64] = cz[:, 2 * c, :]      # k = 2c+1
        kinks[c, 0:64, 64:128] = ch[:, 2 * c, :]
        kinks[c, 64:128, 0:64] = cz[:, 2 * c + 1, :]  # k = 2c+2
        kinks[c, 64:128, 64:128] = ch[:, 2 * c + 1, :]
    return lin, kinks, bz, bh


def _host_prep(vz0, vh0, vz1, vh1, vz2, vh2, vout):
    lins, kinks, betas = [], [], np.zeros((64, 8), np.float32)
    for li, (vz, vh) in enumerate(((vz0, vh0), (vz1, vh1), (vz2, vh2))):
        lin, kk, bz, bh = _layer_tables(vz, vh)
        lins.append(lin)
        kinks.append(kk)
        betas[:, 2 * li] = bz
        betas[:, 2 * li + 1] = bh
    bo, b1o, co = _parts(vout)
    betas[:, 6] = bo
    kko = np.moveaxis(co.reshape(64, 7, 2, 64), 2, 1).reshape(64, 7, 128)
    # kko[i, c, 64r + o] = co[i, 2c+r, o]; we need kinkso[c, 64r+i, o]
    kinkso = np.zeros((7, 128, 64), np.float32)
    for c in range(7):
        kinkso[c, 0:64, :] = co[:, 2 * c, :]
        kinkso[c, 64:128, :] = co[:, 2 * c + 1, :]
    negq = np.zeros((128, 7), np.float32)
    for c in range(7):
        negq[0:64, c] = -(2 * c + 1)
        negq[64:128, c] = -(2 * c + 2)
    return {
        "linL": np.stack(lins, 1),                          # (64, 3, 128)
        "lino": b1o,                                        # (64, 64)
        # (128, 21, 128): partition-major packing of 3 layers x 7 chunks
        "kinksL": np.concatenate(kinks, 0).transpose(1, 0, 2).copy(),
        "kinkso": kinkso.transpose(1, 0, 2).copy(),         # (128, 7, 64)
        "betas": betas,                                     # (64, 8)
        "negq": negq,                                       # (128, 7)
        "ident": np.eye(128, dtype=np.float32),             # (128, 128)
    }


# ---------------------------------------------------------------- bass kernel
def _build_nc():
    nc = bacc.Bacc()
    xT = nc.dram_tensor("xT", [D, T], F32, kind="ExternalInput")
    linL = nc.dram_tensor("linL", [64, 3, 128], F32, kind="ExternalInput")
    lino = nc.dram_tensor("lino", [64, 64], F32, kind="ExternalInput")
    kinksL = nc.dram_tensor("kinksL", [128, 21, 128], F32, kind="ExternalInput")
    kinkso = nc.dram_tensor("kinkso", [128, 7, 64], F32, kind="ExternalInput")
    betas = nc.dram_tensor("betas", [64, 8], F32, kind="ExternalInput")
    negq = nc.dram_tensor("negq", [128, 7], F32, kind="ExternalInput")
    ident = nc.dram_tensor("ident", [128, 128], F32, kind="ExternalInput")
    outT = nc.dram_tensor("outT", [D, T], F32, kind="ExternalOutput")

    with TileContext(nc) as tc:
        with (
            tc.tile_pool(name="singles", bufs=1) as singles,
            tc.tile_pool(name="stack2p", bufs=2) as stack2p,
            tc.tile_pool(name="wpool", bufs=1) as wpool,
            tc.tile_pool(name="seq", bufs=2) as seqp,
            tc.tile_pool(name="nmaj", bufs=2) as nmaj,
            tc.tile_pool(name="small", bufs=2) as small,
            tc.tile_pool(name="outp", bufs=1) as outp,
            tc.tile_pool(name="mmps", bufs=2, space="PSUM") as mmps,
            tc.tile_pool(name="trps", bufs=2, space="PSUM") as trps,
            tc.tile_pool(name="s2ps", bufs=2, space="PSUM") as s2ps,
        ):
            lin_sb = singles.tile([64, 3, 128], F32)
            nc.sync.dma_start(out=lin_sb, in_=linL[:, :, :])
            lino_sb = singles.tile([64, 64], F32)
            nc.sync.dma_start(out=lino_sb, in_=lino[:, :])
            kinks_sb = singles.tile([128, 21, 128], F32)
            nc.sync.dma_start(out=kinks_sb, in_=kinksL[:, :, :])
            kinkso_sb = singles.tile([128, 7, 64], F32)
            nc.sync.dma_start(out=kinkso_sb, in_=kinkso[:, :, :])
            betas_sb = singles.tile([64, 8], F32)
            nc.sync.dma_start(out=betas_sb, in_=betas[:, :])
            negq_sb = singles.tile([128, 7], F32)
            nc.sync.dma_start(out=negq_sb, in_=negq[:, :])
            ident_sb = singles.tile([128, 128], F32)
            nc.sync.dma_start(out=ident_sb, in_=ident[:, :])

            # layer-0 stack2: [x^T; x^T] then t = (clip(x,-1,1)+1)*7.5 in place
            stack2 = stack2p.tile([128, T], F32, tag="stack2")
            nc.sync.dma_start(out=stack2[0:64, :], in_=xT[:, :])
            nc.sync.dma_start(out=stack2[64:128, :], in_=xT[:, :])
            nc.vector.tensor_scalar(stack2, stack2, -1.0, 1.0, Alu.max, Alu.min)
            nc.vector.tensor_scalar(stack2, stack2, 7.5, 7.5, Alu.mult, Alu.add)

            def build_w(stack2_sb):
                """7 relu-kink chunks, split across DVE / GPSIMD / ACT."""
                ws = []
                for c in range(7):
                    w = wpool.tile([128, T], F32, tag=f"W{c}")
                    if c < 3:
                        nc.vector.tensor_scalar(
                            w, stack2_sb, negq_sb[:, c : c + 1], 0.0, Alu.add, Alu.max
                        )
                    elif c < 5:
                        nc.gpsimd.tensor_scalar(
                            w, stack2_sb, negq_sb[:, c : c + 1], 0.0, Alu.add, Alu.max
                        )
                    else:
                        nc.scalar.activation(
                            w, stack2_sb, Act.Relu, bias=negq_sb[:, c : c + 1], scale=1.0
                        )
                    ws.append(w)
                return ws

            def mm_unit(stack2_sb, ws, li):
                """One layer's stacked [z|h] matmul; returns z, b tiles (64,T)."""
                z = seqp.tile([64, T], F32, tag="z")
                b = seqp.tile([64, T], F32, tag="b")
                a = seqp.tile([64, T], F32, tag="a")
                for w in range(NW):
                    sl = slice(w * WIN, (w + 1) * WIN)
                    ps = mmps.tile([128, WIN], F32, tag="mm")
                    nc.tensor.matmul(
                        ps, lin_sb[:, li, :], stack2_sb[0:64, sl], start=True, stop=False
                    )
                    for c in range(7):
                        nc.tensor.matmul(
                            ps,
                            kinks_sb[:, li * 7 + c, :],
                            ws[c][:, sl],
                            start=False,
                            stop=(c == 6),
                        )
                    nc.scalar.activation(
                        z[:, sl], ps[0:64, :], Act.Sigmoid,
                        bias=betas_sb[:, 2 * li : 2 * li + 1], scale=1.0,
                    )
                    # b = (hbar_psum + beta_h) * z
                    nc.vector.scalar_tensor_tensor(
                        b[:, sl], ps[64:128, :], betas_sb[:, 2 * li + 1 : 2 * li + 2],
                        z[:, sl], Alu.add, Alu.mult,
                    )
                    nc.gpsimd.tensor_scalar(a[:, sl], z[:, sl], -1.0, 1.0, Alu.mult, Alu.add)
                return a, b

            def scan_norm(a, b):
                """h = scan(a,b); returns next stack2 (128,T) with t=(hn+1)*7.5."""
                hT = seqp.tile([64, T], F32, tag="hT")
                for w in range(NW):
                    sl = slice(w * WIN, (w + 1) * WIN)
                    init = 0.0 if w == 0 else hT[:, w * WIN - 1 : w * WIN]
                    nc.vector.tensor_tensor_scan(
                        hT[:, sl], a[:, sl], b[:, sl], init, Alu.mult, Alu.add
                    )
                # transpose h^T -> n-major
                hnm = nmaj.tile([128, NT, 64], F32, tag="hnm")
                for w in range(NW):
                    tp = trps.tile([128, 4, 64], F32, tag="htr")
                    for j in range(4):
                        nn = w * 4 + j
                        nc.tensor.transpose(
                            tp[:, j, :],
                            hT[:, nn * 128 : (nn + 1) * 128],
                            ident_sb[0:64, 0:64],
                        )
                    nc.scalar.activation(hnm[:, 4 * w : 4 * w + 4, :], tp, Act.Copy)
                s = small.tile([128, NT], F32, tag="s")
                nc.vector.tensor_reduce(
                    s, hnm, mybir.AxisListType.X, Alu.max, apply_absolute_value=True
                )
                nc.vector.tensor_scalar(s, s, EPS, 1.0 / 7.5, Alu.add, Alu.mult)
                r2 = small.tile([128, NT], F32, tag="r2")
                nc.vector.reciprocal(r2, s)
                tnm = nmaj.tile([128, NT, 64], F32, tag="tnm")
                for j in range(NT):
                    nc.vector.tensor_scalar(
                        tnm[:, j, :], hnm[:, j, :], r2[:, j : j + 1], 7.5,
                        Alu.mult, Alu.add,
                    )
                # transpose t -> stack2[0:64]; DMA-dup to stack2[64:128]
                s2 = stack2p.tile([128, T], F32, tag="stack2")
                for w in range(NW):
                    ps = s2ps.tile([64, WIN], F32, tag="s2")
                    for j in range(4):
                        nn = w * 4 + j
                        nc.tensor.transpose(
                            ps[:, j * 128 : (j + 1) * 128], tnm[:, nn, :], ident_sb
                        )
                    nc.scalar.activation(s2[0:64, w * WIN : (w + 1) * WIN], ps, Act.Copy)
                nc.sync.dma_start(out=s2[64:128, :], in_=s2[0:64, :])
                return s2

            for li in range(3):
                ws = build_w(stack2)
                a, b = mm_unit(stack2, ws, li)
                stack2 = scan_norm(a, b)

            # output APL (M=64)
            ws = build_w(stack2)
            outT_sb = outp.tile([64, T], F32)
            for w in range(NW):
                sl = slice(w * WIN, (w + 1) * WIN)
                psf = mmps.tile([128, WIN], F32, tag="mm")
                ps = psf[0:64, :]
                nc.tensor.matmul(ps, lino_sb, stack2[0:64, sl], start=True, stop=False)
                for c in range(7):
                    nc.tensor.matmul(
                        ps, kinkso_sb[:, c, :], ws[c][:, sl], start=False, stop=(c == 6)
                    )
                nc.scalar.activation(
                    outT_sb[:, sl], ps, Act.Identity, bias=betas_sb[:, 6:7], scale=1.0
                )
            nc.sync.dma_start(out=outT[:, :], in_=outT_sb)

    nc.compile()
    return nc


_NC_CACHE = None


def _get_nc():
    global _NC_CACHE
    if _NC_CACHE is None:
        _NC_CACHE = _build_nc()
    return _NC_CACHE


def kernel(x, vz0, vh0, vz1, vh1, vz2, vh2, vout):
    x = np.ascontiguousarray(np.asarray(x, np.float32))
    aux = _host_prep(vz0, vh0, vz1, vh1, vz2, vh2, vout)
    nc = _get_nc()
    in_maps = []
    for core in range(NCORES):
        m = dict(aux)
        m["xT"] = np.ascontiguousarray(x[core].T)
        in_maps.append(m)
    res = run_bass_kernel_spmd(nc, in_maps, list(range(NCORES)))
    out = np.stack([np.asarray(r["outT"]).T for r in res.results])
    return out.astype(np.float32)


if __name__ == "__main__":
    rng = np.random.default_rng(0)
    ins = {
        "x": rng.standard_normal((8, T, D), dtype=np.float32),
        **{
            n: 0.5 * rng.standard_normal((64, 16, 64), dtype=np.float32)
            for n in ("vz0", "vh0", "vz1", "vh1", "vz2", "vh2", "vout")
        },
    }
    print(kernel(**ins).shape)


# revision 13
# speedup vs baseline: 1.2631x; 1.2631x over previous
"""Trainium2 Bass kernel for nn_MinGRUStack (B=8, T=2048, Din=D=Dout=64, P=16).

Strategy (data-parallel over B, one sequence per NeuronCore, SBUF-resident):

  Each APL layer `out[n,o] = sum_i pwl_interp(t[n,i]; v[i,:,o])` is rewritten
  exactly in the relu-kink basis:
      f(t) = v0 + s0*t + sum_{k=1..14} (s_k - s_{k-1}) * relu(t - k)
  so each APL becomes a K=15*64 matmul chain on the PE:
      u^T = lin^T @ t^T + sum_c kink_c^T @ relu(stack2 - (2c+1))
  where stack2 = [t^T; t^T - 1] (128 partitions): the second copy is
  pre-shifted by -1 so one 128-partition tensor_scalar op with an IMMEDIATE
  scalar builds two relu chunks at once (per-partition AP scalars are ~20x
  slower on DVE/GPSIMD). z and h_bar share the basis: tables stacked into one
  M=128 matmul ([z outs | h outs]); the constant terms fold into the
  PSUM-eviction activation bias.

  The MinGRU recurrence h[t] = (1-z)h[t-1] + z*h_bar runs as a native DVE
  tensor_tensor_scan (stable direct form). max-abs normalization and the next
  layer's t = (clip(hn)+1)*7.5 fold into one scalar_tensor_tensor per window
  using r2 = 7.5/(s+eps) (clip is a no-op since |h/s| <= 1).

  Table preprocessing (second differences of the tiny value tables, layout
  packing) happens on the host; the device reads the same table bytes.
"""

import numpy as np

import concourse.bass as bass
import concourse.bacc as bacc
from concourse import mybir
from concourse.tile import TileContext
from concourse.bass_utils import run_bass_kernel_spmd

F32 = mybir.dt.float32
Alu = mybir.AluOpType
Act = mybir.ActivationFunctionType

T, D, NCORES = 2048, 64, 8
NW, WIN = 4, 512          # matmul windows
NT = T // 128             # 16 transpose chunks
EPS = 1e-6


# ---------------------------------------------------------------- host tables
def _parts(v):
    v = np.asarray(v, np.float32)
    beta = v[:, 0, :].sum(0)                      # (64,)
    b1 = v[:, 1, :] - v[:, 0, :]                  # (64, 64)
    s = v[:, 1:, :] - v[:, :-1, :]                # (64, 15, 64)
    c = s[:, 1:, :] - s[:, :-1, :]                # (64, 14, 64)  k=1..14
    return beta.astype(np.float32), b1.astype(np.float32), c.astype(np.float32)


def _layer_tables(vz, vh):
    bz, b1z, cz = _parts(vz)
    bh, b1h, ch = _parts(vh)
    lin = np.concatenate([b1z, b1h], axis=1)      # (64, 128)
    kinks = np.zeros((7, 128, 128), np.float32)
    for c in range(7):
        kinks[c, 0:64, 0:64] = cz[:, 2 * c, :]      # k = 2c+1
        kinks[c, 0:64, 64:128] = ch[:, 2 * c, :]
        kinks[c, 64:128, 0:64] = cz[:, 2 * c + 1, :]  # k = 2c+2
        kinks[c, 64:128, 64:128] = ch[:, 2 * c + 1, :]
    return lin, kinks, bz, bh


def _host_prep(vz0, vh0, vz1, vh1, vz2, vh2, vout):
    lins, kinks, betas = [], [], np.zeros((64, 8), np.float32)
    for li, (vz, vh) in enumerate(((vz0, vh0), (vz1, vh1), (vz2, vh2))):
        lin, kk, bz, bh = _layer_tables(vz, vh)
        lins.append(lin)
        kinks.append(kk)
        betas[:, 2 * li] = bz
        betas[:, 2 * li + 1] = bh
    bo, b1o, co = _parts(vout)
    betas[:, 6] = bo
    kinkso = np.zeros((7, 128, 64), np.float32)
    for c in range(7):
        kinkso[c, 0:64, :] = co[:, 2 * c, :]
        kinkso[c, 64:128, :] = co[:, 2 * c + 1, :]
    return {
        "linL": np.stack(lins, 1),                          # (64, 3, 128)
        "lino": b1o,                                        # (64, 64)
        # (128, 21, 128): partition-major packing of 3 layers x 7 chunks
        "kinksL": np.concatenate(kinks, 0).transpose(1, 0, 2).copy(),
        "kinkso": kinkso.transpose(1, 0, 2).copy(),         # (128, 7, 64)
        "betas": betas,                                     # (64, 8)
        "negq": np.repeat(-np.arange(1.0, 15.0, 2.0, dtype=np.float32)[None, :], 128, 0),  # (128, 7)
        "ident": np.eye(128, dtype=np.float32),             # (128, 128)
    }


# ---------------------------------------------------------------- bass kernel
def _build_nc():
    nc = bacc.Bacc()
    xT2 = nc.dram_tensor("xT2", [128, T], F32, kind="ExternalInput")
    linL = nc.dram_tensor("linL", [64, 3, 128], F32, kind="ExternalInput")
    lino = nc.dram_tensor("lino", [64, 64], F32, kind="ExternalInput")
    kinksL = nc.dram_tensor("kinksL", [128, 21, 128], F32, kind="ExternalInput")
    kinkso = nc.dram_tensor("kinkso", [128, 7, 64], F32, kind="ExternalInput")
    betas = nc.dram_tensor("betas", [64, 8], F32, kind="ExternalInput")
    negq = nc.dram_tensor("negq", [128, 7], F32, kind="ExternalInput")
    ident = nc.dram_tensor("ident", [128, 128], F32, kind="ExternalInput")
    outT = nc.dram_tensor("outT", [D, T], F32, kind="ExternalOutput")

    with TileContext(nc) as tc:
        with (
            tc.tile_pool(name="singles", bufs=1) as singles,
            tc.tile_pool(name="stack2p", bufs=2) as stack2p,
            tc.tile_pool(name="wpool", bufs=1) as wpool,
            tc.tile_pool(name="seq", bufs=2) as seqp,
            tc.tile_pool(name="nmaj", bufs=2) as nmaj,
            tc.tile_pool(name="small", bufs=2) as small,
            tc.tile_pool(name="outp", bufs=1) as outp,
            tc.tile_pool(name="mmps", bufs=4, space="PSUM") as mmps,
            tc.tile_pool(name="trps", bufs=2, space="PSUM") as trps,
            tc.tile_pool(name="s2ps", bufs=2, space="PSUM") as s2ps,
        ):
            lin_sb = singles.tile([64, 3, 128], F32)
            nc.sync.dma_start(out=lin_sb, in_=linL[:, :, :])
            lino_sb = singles.tile([64, 64], F32)
            nc.sync.dma_start(out=lino_sb, in_=lino[:, :])
            kinks_sb = singles.tile([128, 21, 128], F32)
            nc.sync.dma_start(out=kinks_sb, in_=kinksL[:, :, :])
            kinkso_sb = singles.tile([128, 7, 64], F32)
            nc.sync.dma_start(out=kinkso_sb, in_=kinkso[:, :, :])
            betas_sb = singles.tile([64, 8], F32)
            nc.sync.dma_start(out=betas_sb, in_=betas[:, :])
            negq_sb = singles.tile([128, 7], F32)
            nc.sync.dma_start(out=negq_sb, in_=negq[:, :])
            ident_sb = singles.tile([128, 128], F32)
            nc.sync.dma_start(out=ident_sb, in_=ident[:, :])
            seven5 = singles.tile([128, 64], F32)
            nc.vector.memset(seven5, 7.5)

            # layer-0 stack2: [x^T; x^T] -> clip -> [t^T; t^T - 1]
            stack2 = stack2p.tile([128, T], F32, tag="stack2")
            nc.sync.dma_start(out=stack2, in_=xT2[:, :])
            nc.vector.tensor_scalar(stack2, stack2, -1.0, 1.0, Alu.max, Alu.min)
            nc.vector.tensor_scalar(
                stack2[0:64, :], stack2[0:64, :], 7.5, 7.5, Alu.mult, Alu.add
            )
            nc.gpsimd.tensor_scalar(
                stack2[64:128, :], stack2[64:128, :], 7.5, 6.5, Alu.mult, Alu.add
            )

            def build_w(stack2_sb):
                """7 relu-kink chunks with immediate scalars, DVE/GPS/ACT."""
                ws = []
                for c in range(7):
                    w = wpool.tile([128, T], F32, tag=f"W{c}")
                    q = -float(2 * c + 1)
                    if c in (0, 1, 2):
                        nc.vector.tensor_scalar(w, stack2_sb, q, 0.0, Alu.add, Alu.max)
                    elif c in (3, 4):
                        nc.gpsimd.tensor_scalar(w, stack2_sb, q, 0.0, Alu.add, Alu.max)
                    else:
                        nc.scalar.activation(
                            w, stack2_sb, Act.Relu, bias=negq_sb[:, c : c + 1], scale=1.0
                        )
                    ws.append(w)
                return ws

            def mm_unit(stack2_sb, ws, li):
                """One layer's stacked [z|h] matmul; weight-stationary order."""
                z = seqp.tile([64, T], F32, tag="z")
                b = seqp.tile([64, T], F32, tag="b")
                a = seqp.tile([64, T], F32, tag="a")
                pss = [mmps.tile([128, WIN], F32, tag="mm", name=f"mmps{w}") for w in range(NW)]
                for w in range(NW):
                    sl = slice(w * WIN, (w + 1) * WIN)
                    nc.tensor.matmul(
                        pss[w], lin_sb[:, li, :], stack2_sb[0:64, sl],
                        start=True, stop=False,
                    )
                for c in range(7):
                    for w in range(NW):
                        sl = slice(w * WIN, (w + 1) * WIN)
                        nc.tensor.matmul(
                            pss[w], kinks_sb[:, li * 7 + c, :], ws[c][:, sl],
                            start=False, stop=(c == 6),
                        )
                for w in range(NW):
                    sl = slice(w * WIN, (w + 1) * WIN)
                    nc.scalar.activation(
                        z[:, sl], pss[w][0:64, :], Act.Sigmoid,
                        bias=betas_sb[:, 2 * li : 2 * li + 1], scale=1.0,
                    )
                    # b = (hbar_psum + beta_h) * z
                    nc.vector.scalar_tensor_tensor(
                        b[:, sl], pss[w][64:128, :],
                        betas_sb[:, 2 * li + 1 : 2 * li + 2],
                        z[:, sl], Alu.add, Alu.mult,
                    )
                    nc.gpsimd.tensor_scalar(
                        a[:, sl], z[:, sl], -1.0, 1.0, Alu.mult, Alu.add
                    )
                return a, b

            def scan_norm(a, b):
                """h = scan(a,b); returns next stack2 = [t^T; t^T - 1]."""
                hT = seqp.tile([64, T], F32, tag="hT")
                for w in range(NW):
                    sl = slice(w * WIN, (w + 1) * WIN)
                    init = 0.0 if w == 0 else hT[:, w * WIN - 1 : w * WIN]
                    nc.vector.tensor_tensor_scan(
                        hT[:, sl], a[:, sl], b[:, sl], init, Alu.mult, Alu.add
                    )
                # transpose h^T -> n-major
                hnm = nmaj.tile([128, NT, 64], F32, tag="hnm")
                for w in range(NW):
                    tp = trps.tile([128, 4, 64], F32, tag="htr")
                    for j in range(4):
                        nn = w * 4 + j
                        nc.tensor.transpose(
                            tp[:, j, :],
                            hT[:, nn * 128 : (nn + 1) * 128],
                            ident_sb[0:64, 0:64],
                        )
                    nc.scalar.activation(hnm[:, 4 * w : 4 * w + 4, :], tp, Act.Copy)
                s = small.tile([128, NT], F32, tag="s")
                nc.vector.tensor_reduce(
                    s, hnm, mybir.AxisListType.X, Alu.max, apply_absolute_value=True
                )
                nc.vector.tensor_scalar(s, s, EPS, 1.0 / 7.5, Alu.add, Alu.mult)
                r2 = small.tile([128, NT], F32, tag="r2")
                nc.vector.reciprocal(r2, s)
                tnm = nmaj.tile([128, NT, 64], F32, tag="tnm")
                for j in range(NT):
                    # t = hnm * r2 + 7.5 (const tile avoids slow AP-scalar ts)
                    nc.vector.scalar_tensor_tensor(
                        tnm[:, j, :], hnm[:, j, :], r2[:, j : j + 1], seven5,
                        Alu.mult, Alu.add,
                    )
                # transpose t -> stack2[0:64]; shifted copy -> stack2[64:128]
                s2 = stack2p.tile([128, T], F32, tag="stack2")
                for w in range(NW):
                    ps = s2ps.tile([64, WIN], F32, tag="s2")
                    for j in range(4):
                        nn = w * 4 + j
                        nc.tensor.transpose(
                            ps[:, j * 128 : (j + 1) * 128], tnm[:, nn, :], ident_sb
                        )
                    sl = slice(w * WIN, (w + 1) * WIN)
                    nc.scalar.activation(s2[0:64, sl], ps, Act.Copy)
                    nc.scalar.activation(
                        s2[64:128, sl], ps, Act.Identity,
                        bias=negq_sb[0:64, 0:1], scale=1.0,
                    )
                return s2

            for li in range(3):
                ws = build_w(stack2)
                a, b = mm_unit(stack2, ws, li)
                stack2 = scan_norm(a, b)

            # output APL (M=64)
            ws = build_w(stack2)
            outT_sb = outp.tile([64, T], F32)
            pso = [mmps.tile([128, WIN], F32, tag="mm", name=f"mmpso{w}") for w in range(NW)]
            for w in range(NW):
                sl = slice(w * WIN, (w + 1) * WIN)
                nc.tensor.matmul(
                    pso[w][0:64, :], lino_sb, stack2[0:64, sl], start=True, stop=False
                )
            for c in range(7):
                for w in range(NW):
                    sl = slice(w * WIN, (w + 1) * WIN)
                    nc.tensor.matmul(
                        pso[w][0:64, :], kinkso_sb[:, c, :], ws[c][:, sl],
                        start=False, stop=(c == 6),
                    )
            for w in range(NW):
                sl = slice(w * WIN, (w + 1) * WIN)
                nc.scalar.activation(
                    outT_sb[:, sl], pso[w][0:64, :], Act.Identity,
                    bias=betas_sb[:, 6:7], scale=1.0,
                )
            nc.sync.dma_start(out=outT[:, :], in_=outT_sb)

    nc.compile()
    return nc


_NC_CACHE = None


def _get_nc():
    global _NC_CACHE
    if _NC_CACHE is None:
        _NC_CACHE = _build_nc()
    return _NC_CACHE


def kernel(x, vz0, vh0, vz1, vh1, vz2, vh2, vout):
    x = np.ascontiguousarray(np.asarray(x, np.float32))
    aux = _host_prep(vz0, vh0, vz1, vh1, vz2, vh2, vout)
    nc = _get_nc()
    in_maps = []
    for core in range(NCORES):
        m = dict(aux)
        xt = np.ascontiguousarray(x[core].T)                 # (64, T)
        m["xT2"] = np.concatenate([xt, xt], axis=0)          # [x^T; x^T]
        in_maps.append(m)
    res = run_bass_kernel_spmd(nc, in_maps, list(range(NCORES)))
    out = np.stack([np.asarray(r["outT"]).T for r in res.results])
    return out.astype(np.float32)


if __name__ == "__main__":
    rng = np.random.default_rng(0)
    ins = {
        "x": rng.standard_normal((8, T, D), dtype=np.float32),
        **{
            n: 0.5 * rng.standard_normal((64, 16, 64), dtype=np.float32)
            for n in ("vz0", "vh0", "vz1", "vh1", "vz2", "vh2", "vout")
        },
    }
    print(kernel(**ins).shape)
